# revision 1
# baseline (speedup 1.0000x reference)
import sys

for _p in ("/opt/trn_rl_repo",):
    if _p not in sys.path:
        sys.path.insert(0, _p)

import numpy as np
import bass_rust
import concourse.bass as bass
import concourse.mybir as mybir
import concourse.tile as tile
from concourse.bass_utils import run_bass_kernel_spmd

DT = mybir.dt

# Problem constants (hardcoded from the nn_AutoFlow spec)
B, D, NH0, NH1, L = 32768, 64, 256, 256, 16
NCORES = 8
BC = B // NCORES          # 4096 samples per core
BT = 512                  # batch tile (free dim of activation tiles)
NT = BC // BT             # tiles per core
NCH = 512                 # matmul moving-operand chunk (one PSUM bank fp32)
NC_PER_T = BT // NCH      # chunks per tile
PS_BUFS_HP = 6
WARM_MMS = 40
WARM_N = 128
PS_BUFS_LP = 2

# per-layer fp16 weight blob column layout
O_A0 = {0: 0, 1: 256}                       # [64,256] W0'^T per net
O_A1 = {0: (512, 768), 1: (1024, 1280)}     # [128,256] x2 k-chunks per net
O_A2 = {0: (1536, 1600), 1: (1664, 1728)}   # [128,64] x2 k-chunks per net
O_B2L = 1792                                # [1,64] -b2l row (partition 0)
CW = 1856                                   # fp16 cols per layer

# fp32 bias blob column layout (per layer, 10 cols)
CB = 10

# which of the 8 relus per (layer,tile) run on ACT vs DVE
# key: (net, hidden_layer, m_chunk) -> True = ACT; alternated by layer parity
RELU_ON_ACT = {
    0: {
        (0, 0, 0): True, (0, 0, 1): True,
        (1, 0, 0): True, (1, 0, 1): True,
        (0, 1, 0): True, (0, 1, 1): False,
        (1, 1, 0): False, (1, 1, 1): False,
    },
    1: {
        (0, 0, 0): True, (0, 0, 1): True,
        (1, 0, 0): True, (1, 0, 1): True,
        (0, 1, 0): True, (0, 1, 1): False,
        (1, 1, 0): False, (1, 1, 1): False,
    },
}
RELU_SPLIT = {(0, 1, 0)}  # relu split across ACT/DVE by free dim
RELU_SPLIT_ACT = 236


def _patch_tile_drain(maxw=1):
    """walrus on this stack allows only 1 sync-wait on the kernel-tail Drain;
    split the TileContext drain's waits across a chain of drains."""
    from concourse.tile import ScopedClock

    def _drain_and_barrier(self, tick_clock, wait_clock):
        drain_inst = self.nc.sync.drain()
        wait_clock.add_sem_waits(
            drain_inst.ins, ScopedClock({None: tick_clock.global_clock})
        )
        inst = drain_inst.ins
        si = inst.sync_info
        if si is not None:
            waits = list(si.on_wait)
            ups = list(si.on_update)
            if len(waits) > maxw:
                chunks = [waits[i:i + maxw] for i in range(0, len(waits), maxw)]
                inst.sync_info = bass_rust.SyncInfo(on_wait=chunks[0], on_update=[])
                for j, chunk in enumerate(chunks[1:]):
                    extra = self.nc.sync.drain().ins
                    is_last = j == len(chunks) - 2
                    extra.sync_info = bass_rust.SyncInfo(
                        on_wait=chunk, on_update=ups if is_last else []
                    )
        self.nc.all_engine_barrier()
        assert self.sems is not None
        popped = self.nc._tile_sem_poison_stack.pop()
        assert popped is self._sem_poison
        self.nc.clear_and_free_semaphores(list(self.sems.allocated().values()))
        self.nc.all_engine_barrier()

    tile.TileContext._drain_and_barrier = _drain_and_barrier


_MAXW1_TYPES = ("InstDrain", "InstActivation")


def _split_excess_waits(nc, maxw=1):
    """walrus on this stack encodes very few semaphore-wait slots per
    instruction. Spill excess waits onto same-engine NoOps inserted just
    before the instruction (engine streams are in-order, so this is
    equivalent)."""
    for f in nc.m.functions:
        for bb in f.blocks:
            il = bb.instructions
            out = []
            for inst in il:
                si = getattr(inst, "sync_info", None)
                mw = 1 if type(inst).__name__ in _MAXW1_TYPES else maxw
                if si is not None and len(si.on_wait) > mw:
                    waits = list(si.on_wait)
                    ups = list(si.on_update)
                    chunks = [waits[i:i + mw] for i in range(0, len(waits), mw)]
                    for j, ch in enumerate(chunks[:-1]):
                        nop = mybir.InstNoOp(
                            name=f"{inst.name}-wsp{j}", ins=[], outs=[]
                        )
                        nop.engine = inst.engine
                        nop.sync_info = bass_rust.SyncInfo(on_wait=ch, on_update=[])
                        nc.register_instruction(nop, overwrite=True)
                        out.append(nop)
                    inst.sync_info = bass_rust.SyncInfo(
                        on_wait=chunks[-1], on_update=ups
                    )
                out.append(inst)
            if len(out) != len(il):
                il[:] = out


def _build_masks():
    mh0 = np.arange(NH0) % (D - 1)
    mh1 = np.arange(NH1) % (D - 1)
    M1 = (mh0[None, :] <= mh1[:, None]).astype(np.float32)
    M0s, M2s = [], []
    for l in range(L):
        perm = np.arange(D) if l % 2 == 0 else np.arange(D)[::-1]
        M0s.append((perm[None, :] <= mh0[:, None]).astype(np.float32))
        M2s.append((mh1[None, :] < perm[:, None]).astype(np.float32))
    return np.stack(M0s), np.broadcast_to(M1, (L,) + M1.shape).copy(), np.stack(M2s)


PRIO_GROUP = 8


def _PRIO(l, ph, t):
    g, tin = t // PRIO_GROUP, t % PRIO_GROUP
    return (((l * (NT // PRIO_GROUP) + g) * 6 + ph) * PRIO_GROUP + tin) * 64


def _emit_layer(nc, tc, pools, wtile, btile, ones, l, t_idx, y32t, y16t, last):
    f16, f32 = DT.float16, DT.float32
    hpool, lppool, ypool32, ypool16, epool, pspool = pools
    base = l * CW
    bb = l * CB
    AF = mybir.ActivationFunctionType
    ALU = mybir.AluOpType

    h1s = {}   # net -> [h1 chunk0 sbuf, chunk1 sbuf]
    outp = {}  # net -> [64,BT] psum (loc: -(loc+b2l), sc: raw)

    # phase 0: mm0 for both nets
    tc.cur_priority = _PRIO(l, 0, t_idx)
    h0p = {}
    for net in (0, 1):
        h0p[net] = []
        for m in (0, 1):
            p = pspool.tile([128, BT], f32, tag="hp")
            lhsT = wtile[0:64, base + O_A0[net] + 128 * m: base + O_A0[net] + 128 * (m + 1)]
            for c in range(NC_PER_T):
                nc.tensor.matmul(
                    p[:, c * NCH:(c + 1) * NCH],
                    lhsT, y16t[0:64, c * NCH:(c + 1) * NCH],
                    start=True, stop=True,
                )
            h0p[net].append(p)

    # phase 1: relu0 for both nets
    tc.cur_priority = _PRIO(l, 1, t_idx)
    h0s = {}
    for net in (0, 1):
        h0s[net] = []
        for m in (0, 1):
            hs = hpool.tile([128, BT], f16, tag="h")
            bias_ap = btile[:, bb + net * 4 + m: bb + net * 4 + m + 1]
            if RELU_ON_ACT[l % 2][(net, 0, m)]:
                nc.scalar.activation(hs[:], h0p[net][m][:], AF.Relu, bias=bias_ap)
            else:
                nc.vector.tensor_scalar(hs[:], h0p[net][m][:], bias_ap, 0.0,
                                        ALU.add, ALU.max)
            h0s[net].append(hs)

    # phase 2: mm1 for both nets
    tc.cur_priority = _PRIO(l, 2, t_idx)
    h1p = {}
    for net in (0, 1):
        h1p[net] = []
        for m in (0, 1):
            p = pspool.tile([128, BT], f32, tag="hp")
            for c in range(NC_PER_T):
                for k in (0, 1):
                    a1 = O_A1[net][k]
                    lhsT = wtile[0:128, base + a1 + 128 * m: base + a1 + 128 * (m + 1)]
                    nc.tensor.matmul(
                        p[:, c * NCH:(c + 1) * NCH],
                        lhsT, h0s[net][k][0:128, c * NCH:(c + 1) * NCH],
                        start=(k == 0), stop=(k == 1),
                    )
            h1p[net].append(p)

    # phase 3: relu1 for both nets
    tc.cur_priority = _PRIO(l, 3, t_idx)
    for net in (0, 1):
        h1s[net] = []
        for m in (0, 1):
            hs = hpool.tile([128, BT], f16, tag="h")
            bias_ap = btile[:, bb + net * 4 + 2 + m: bb + net * 4 + 2 + m + 1]
            if (net, 1, m) in RELU_SPLIT:
                hb = RELU_SPLIT_ACT
                nc.scalar.activation(hs[:, 0:hb], h1p[net][m][:, 0:hb],
                                     AF.Relu, bias=bias_ap)
                nc.vector.tensor_scalar(hs[:, hb:BT], h1p[net][m][:, hb:BT],
                                        bias_ap, 0.0, ALU.add, ALU.max)
            elif RELU_ON_ACT[l % 2][(net, 1, m)]:
                nc.scalar.activation(hs[:], h1p[net][m][:], AF.Relu, bias=bias_ap)
            else:
                nc.vector.tensor_scalar(hs[:], h1p[net][m][:], bias_ap, 0.0,
                                        ALU.add, ALU.max)
            h1s[net].append(hs)

    # phase 4: mm2 for both nets
    tc.cur_priority = _PRIO(l, 4, t_idx)
    for net in (0, 1):
        op = lppool.tile([64, BT], f32, tag="lp")
        h1list = h1s[net]
        for c in range(NC_PER_T):
            cs = slice(c * NCH, (c + 1) * NCH)
            first = True
            for k in (0, 1):
                a2 = O_A2[net][k]
                nc.tensor.matmul(
                    op[:, cs],
                    wtile[0:128, base + a2: base + a2 + 64],
                    h1list[k][0:128, cs],
                    start=first, stop=(k == 1),
                )
                first = False
        outp[net] = op

    # ---- coupling: y' = exp(-(sc+b2s)) * (y32 - loc - b2l) ----
    tc.cur_priority = _PRIO(l, 5, t_idx)
    e32 = epool.tile([64, BT], DT.float32, tag="e")
    t32 = epool.tile([64, BT], DT.float32, tag="t")
    ny16 = None
    nc.scalar.activation(
        e32[:], outp[1][:], AF.Exp,
        bias=btile[0:64, bb + 8: bb + 9], scale=-1.0,
    )
    yb = epool.tile([64, BT], DT.float32, tag="yb")
    nc.gpsimd.tensor_scalar(yb[:], y32t[0:64, :],
                            btile[0:64, bb + 9: bb + 10], None, ALU.add)
    nc.vector.tensor_tensor(t32[:], yb[:], outp[0][:], ALU.add)
    if not last:
        # fp16 copy feeds the next layer's matmuls: keep it off the fp32 path
        ny16 = ypool16.tile([64, BT], DT.float16, tag="y16")
        nc.gpsimd.tensor_tensor(ny16[:], t32[:], e32[:], ALU.mult)
    ny32 = ypool32.tile([64, BT], DT.float32, tag="y32")
    if last and t_idx == NT - 1:
        # final tile's coupling is the kernel tail: DVE is faster here
        nc.vector.tensor_tensor(ny32[:], t32[:], e32[:], ALU.mult)
    else:
        nc.gpsimd.tensor_tensor(ny32[:], t32[:], e32[:], ALU.mult)
    return ny32, ny16


def _build():
    _patch_tile_drain(1)
    from contextlib import ExitStack

    f16, f32 = DT.float16, DT.float32
    nc = bass.Bass(target_bir_lowering=False)
    u32_d = nc.declare_dram_parameter("u32", [64, BC], f32, isOutput=False)
    u16_d = nc.declare_dram_parameter("u16", [64, BC], f16, isOutput=False)
    w_d = nc.declare_dram_parameter("w", [L, 128, CW], f16, isOutput=False)
    b_d = nc.declare_dram_parameter("bias", [128, L * CB], f32, isOutput=False)
    out_d = nc.declare_dram_parameter("out", [64, BC], f32, isOutput=True)

    with tile.TileContext(nc) as tc, ExitStack() as ctx:
        wpool = ctx.enter_context(tc.tile_pool(name="w", bufs=1))
        hpool = ctx.enter_context(tc.tile_pool(name="h", bufs=10))
        ypool32 = ctx.enter_context(tc.tile_pool(name="y32", bufs=12))
        ypool16 = ctx.enter_context(tc.tile_pool(name="y16", bufs=12))
        epool = ctx.enter_context(tc.tile_pool(name="e", bufs=4))
        pspool = ctx.enter_context(tc.tile_pool(name="ps", bufs=PS_BUFS_HP, space="PSUM"))
        lppool = ctx.enter_context(tc.tile_pool(name="lps", bufs=PS_BUFS_LP, space="PSUM"))

        wtile = wpool.tile([128, L * CW], f16)
        btile = wpool.tile([128, L * CB], f32)
        # PE warmup: keep the HAM activity monitor busy while the first
        # DMAs land, so the first real matmuls run at 2.4 GHz instead of 1.2
        warm = wpool.tile([128, 128], f16)
        wps = pspool.tile([128, WARM_N], f32, tag="hp")
        nc.gpsimd.memset(warm[:], 0.0)
        for _ in range(WARM_MMS):
            nc.tensor.matmul(wps[:, 0:WARM_N], warm[:, 0:WARM_N],
                             warm[:, 0:WARM_N], start=True, stop=True)

        nc.sync.dma_start(wtile[:, 0:512], w_d[0][:, 0:512])
        nc.sync.dma_start(btile[:], b_d[:])
        nc.sync.dma_start(wtile[:, 512:CW], w_d[0][:, 512:CW])
        y32 = []
        y16 = []
        for t in range(NT):
            t16 = ypool16.tile([64, BT], f16, tag="y16")
            nc.gpsimd.dma_start(t16[:], u16_d[:, t * BT:(t + 1) * BT])
            y16.append(t16)
        for t in range(NT):
            t32 = ypool32.tile([64, BT], f32, tag="y32")
            nc.sync.dma_start(t32[:], u32_d[:, t * BT:(t + 1) * BT])
            y32.append(t32)
        for l in range(1, L):
            nc.sync.dma_start(wtile[:, l * CW:(l + 1) * CW], w_d[l])

        ones = None
        pools = (hpool, lppool, ypool32, ypool16, epool, pspool)
        for l in range(L):
            for t in range(NT):
                y32[t], y16[t] = _emit_layer(
                    nc, tc, pools, wtile, btile, ones, l, t, y32[t], y16[t],
                    l == L - 1
                )
                if l == L - 1:
                    nc.sync.dma_start(out_d[:, t * BT:(t + 1) * BT], y32[t][:])
    _split_excess_waits(nc, maxw=1)
    return nc


_NC_CACHE = None


def _prep_blobs(inputs):
    M0, M1, M2 = _build_masks()
    w_blob = np.zeros((L, 128, CW), np.float16)
    b_blob = np.zeros((128, L * CB), np.float32)
    for l in range(L):
        for net, name in ((0, "loc"), (1, "scale")):
            W0 = (M0[l] * inputs[f"{name}_W0"][l]).astype(np.float32)
            W1 = (M1[l] * inputs[f"{name}_W1"][l]).astype(np.float32)
            W2 = (M2[l] * inputs[f"{name}_W2"][l]).astype(np.float32)
            b0 = inputs[f"{name}_b0"][l]
            b1 = inputs[f"{name}_b1"][l]
            b2 = inputs[f"{name}_b2"][l]
            A0 = W0.T  # [64,256]
            A1 = W1.T  # [256,256]
            A2 = W2.T  # [256,64]
            if net == 0:
                A2 = -A2
                b_blob[0:64, l * CB + 9] = -b2
            else:
                b_blob[0:64, l * CB + 8] = -b2
            w_blob[l, 0:64, O_A0[net]:O_A0[net] + 256] = A0.astype(np.float16)
            for k in (0, 1):
                w_blob[l, 0:128, O_A1[net][k]:O_A1[net][k] + 256] = \
                    A1[128 * k:128 * (k + 1)].astype(np.float16)
                w_blob[l, 0:128, O_A2[net][k]:O_A2[net][k] + 64] = \
                    A2[128 * k:128 * (k + 1)].astype(np.float16)
            b_blob[:, l * CB + net * 4 + 0] = b0[0:128]
            b_blob[:, l * CB + net * 4 + 1] = b0[128:256]
            b_blob[:, l * CB + net * 4 + 2] = b1[0:128]
            b_blob[:, l * CB + net * 4 + 3] = b1[128:256]
    return w_blob, b_blob


def kernel(**inputs):
    global _NC_CACHE
    inputs = {k: np.asarray(v) for k, v in inputs.items()}
    u = inputs["u"].astype(np.float32)            # [B, 64]
    w_blob, b_blob = _prep_blobs(inputs)

    uT = np.ascontiguousarray(u.T)                # [64, B]
    uT16 = uT.astype(np.float16)

    if _NC_CACHE is None:
        _NC_CACHE = _build()
    nc = _NC_CACHE

    in_maps = []
    for c in range(NCORES):
        sl = slice(c * BC, (c + 1) * BC)
        in_maps.append({
            "u32": np.ascontiguousarray(uT[:, sl]),
            "u16": np.ascontiguousarray(uT16[:, sl]),
            "w": w_blob,
            "bias": b_blob,
        })
    res = run_bass_kernel_spmd(nc, in_maps, core_ids=list(range(NCORES)))
    out = np.empty((64, B), np.float32)
    for c in range(NCORES):
        out[:, c * BC:(c + 1) * BC] = res.results[c]["out"]
    return np.ascontiguousarray(out.T)

